# revision 8
# baseline (speedup 1.0000x reference)
"""Canny edge filter (nms_detection) Trainium2 Bass kernel.

Full inputs: x [128, 512, 512] f32 (plus 1x1 gaussian + sobel kernels, which
are compile-time constants here). Output: [128, 512, 512] f32 binary edges.

Strategy: shard the 128 slices across 8 cores (16 per core). Each slice is
independent (3x3 stencils + per-slice max). All math is done in the
squared-magnitude domain (no sqrt / arctan2 needed):
  - gx, gy via fp32 TensorE matmuls with banded stencil matrices
    (vertical part) and column-shifted access patterns (horizontal part).
  - sqx, sqy via ScalarE Square (exact), msq = sqx + sqy.
  - NMS direction via comparisons: t^2*sqx <= sqy etc. (t = tan 22.5deg).
  - neighbor max via DMA partition-shifted copies of msq + col offsets.
  - per-slice max of msq == per-slice max of NMS'd mag^2 (the argmax always
    survives NMS), so thresholds are computed in pass A.
  - hysteresis: 3x3 box-sum of strong in bf16 (exact for 0/1 data).
"""
import sys
import math
from contextlib import ExitStack

sys.path.insert(0, "/opt/trn_rl_repo")

import numpy as np

import concourse.bass as bass
import concourse.bacc as bacc
import concourse.bass_isa as bass_isa
import concourse.mybir as mybir
import concourse.tile as tile
from concourse import bass_utils

F32 = mybir.dt.float32
BF16 = mybir.dt.bfloat16
U8 = mybir.dt.uint8
ALU = mybir.AluOpType
ACTF = mybir.ActivationFunctionType

D, H, W = 128, 512, 512
N_CORES = 8
D_SH = D // N_CORES

# constants matching the reference's f32 arithmetic boundaries
T2 = np.float32((math.sqrt(2.0) - 1.0) ** 2)          # tan^2(22.5 deg)
CSQ = np.float32(np.float64(np.float32(0.05)) ** 2)    # HIGH_T^2
DSQ = np.float32(np.float64(np.float32(0.01)) ** 2)    # LOW_T^2

EDGE_ROWS = 122  # edge rows produced per full strip (128 partitions - 6 halo)


def _stencil_mats():
    """lhsT matrices [128, 126]: out[q,:] = sum_p lhsT[p,q] * x[p,:].
    out row q corresponds to grid row R0+1+q; band reads x partitions q..q+2."""
    vs = np.zeros((128, 126), np.float32)
    vd = np.zeros((128, 126), np.float32)
    for q in range(126):
        vs[q, q] = 1.0
        vs[q + 1, q] = 2.0
        vs[q + 2, q] = 1.0
        vd[q, q] = -1.0
        vd[q + 2, q] = 1.0
    return vs, -vs, vd, 2.0 * vd


def _strips(h):
    """Strip table: (R0, M, is_first, is_last). R0 = grid row of x partition 0.
    msq rows R0+1 .. R0+M, edge rows R0+3 .. R0+2+edge_cnt."""
    n = max(1, math.ceil(h / EDGE_ROWS))
    out = []
    for s in range(n):
        r0 = EDGE_ROWS * s - 2
        m = min(126, (h + 2) - (EDGE_ROWS * s - 1) + 1)
        out.append((r0, m, s == 0, s == n - 1))
    return out


def build_nc(dsh, h, w):
    """Build the per-core Bass program: x [dsh, h, w] -> out [dsh, h, w]."""
    gw = w + 2          # grid width
    xw = w + 6          # x tile width; x col t <-> grid col t-2
    mw = w + 4          # msq tile width; msq col t <-> grid col t-1
    # column blocks of the msq range [1, gw+1) (grid cols 0..gw-1)
    blocks = []
    c = 1
    while c < gw + 1:
        bw = min(258, gw + 1 - c)
        blocks.append((c, bw))
        c += bw
    strips = _strips(h)

    nc = bacc.Bacc(trn_type="TRN2")
    x_d = nc.dram_tensor("x", [dsh, h, w], F32, kind="ExternalInput")
    vs_d = nc.dram_tensor("vs", [128, 126], F32, kind="ExternalInput")
    vsn_d = nc.dram_tensor("vsn", [128, 126], F32, kind="ExternalInput")
    vd_d = nc.dram_tensor("vd", [128, 126], F32, kind="ExternalInput")
    vd2_d = nc.dram_tensor("vd2", [128, 126], F32, kind="ExternalInput")
    o_d = nc.dram_tensor("o", [dsh, h, w], F32, kind="ExternalOutput")

    with ExitStack() as ctx:
        tc = ctx.enter_context(tile.TileContext(nc))
        consts = ctx.enter_context(tc.tile_pool(name="consts", bufs=1))
        xp = ctx.enter_context(tc.tile_pool(name="xp", bufs=3))
        ps = ctx.enter_context(tc.tile_pool(name="ps", bufs=2, space="PSUM"))
        sqp = ctx.enter_context(tc.tile_pool(name="sqp", bufs=2))
        msqp = ctx.enter_context(tc.tile_pool(name="msqp", bufs=12))
        maskp = ctx.enter_context(tc.tile_pool(name="maskp", bufs=12))
        accp = ctx.enter_context(tc.tile_pool(name="accp", bufs=14))
        thrp = ctx.enter_context(tc.tile_pool(name="thrp", bufs=4))
        udp = ctx.enter_context(tc.tile_pool(name="udp", bufs=2))
        nmaxp = ctx.enter_context(tc.tile_pool(name="nmaxp", bufs=2))
        selp = ctx.enter_context(tc.tile_pool(name="selp", bufs=2))
        bfp = ctx.enter_context(tc.tile_pool(name="bfp", bufs=2))
        outp = ctx.enter_context(tc.tile_pool(name="outp", bufs=3))

        vs_s = consts.tile([128, 126], F32, tag="vs")
        vsn_s = consts.tile([128, 126], F32, tag="vsn")
        vd_s = consts.tile([128, 126], F32, tag="vd")
        vd2_s = consts.tile([128, 126], F32, tag="vd2")
        nc.sync.dma_start(vs_s, vs_d[:])
        nc.sync.dma_start(vsn_s, vsn_d[:])
        nc.sync.dma_start(vd_s, vd_d[:])
        nc.sync.dma_start(vd2_s, vd2_d[:])

        for sl in range(dsh):
            msq_t, m1_t, is90_t, s_t, acc_t = [], [], [], [], []
            # ---------------- pass A ----------------
            for (r0, m, first, last) in strips:
                xt = xp.tile([128, xw], F32, tag="x", name=f"x_{sl}_{r0}")
                # zero pads (compute-engine memsets must start at partition 0/32/64/96,
                # so bottom-pad strips just clear the whole tile first)
                img_lo = max(1, r0) - r0          # first partition with image data
                img_hi = min(h, r0 + 127) - r0    # last partition with image data
                if img_hi < 127:
                    nc.gpsimd.memset(xt[:, :], 0.0)
                else:
                    nc.gpsimd.memset(xt[:, 0:3], 0.0)
                    nc.gpsimd.memset(xt[:, xw - 3:xw], 0.0)
                    if img_lo > 0:
                        nc.gpsimd.memset(xt[0:img_lo, :], 0.0)
                nc.sync.dma_start(
                    xt[img_lo:img_hi + 1, 3:w + 3],
                    x_d[sl, r0 + img_lo - 1:r0 + img_hi, 0:w])

                msq = msqp.tile([128, mw], F32, tag="msq", name=f"msq_{sl}_{r0}")
                sqx = sqp.tile([128, mw], F32, tag="sqx", name=f"sqx_{sl}_{r0}")
                sqy = sqp.tile([128, mw], F32, tag="sqy", name=f"sqy_{sl}_{r0}")
                sgx = sqp.tile([128, mw], BF16, tag="sgx", name=f"sgx_{sl}_{r0}")
                sgy = sqp.tile([128, mw], BF16, tag="sgy", name=f"sgy_{sl}_{r0}")
                for (c0, bw) in blocks:
                    # x tile col of grid col g is g+2; block covers grid cols c0-1 .. c0-1+bw-1
                    xl, xc, xr = c0, c0 + 1, c0 + 2
                    gx = ps.tile([126, 258], F32, tag="gx", name=f"gx_{sl}_{r0}_{c0}")
                    gy = ps.tile([126, 258], F32, tag="gy", name=f"gy_{sl}_{r0}_{c0}")
                    nc.tensor.matmul(gx[0:m, 0:bw], vsn_s[:, 0:m], xt[:, xl:xl + bw],
                                     start=True, stop=False)
                    nc.tensor.matmul(gx[0:m, 0:bw], vs_s[:, 0:m], xt[:, xr:xr + bw],
                                     start=False, stop=True)
                    nc.tensor.matmul(gy[0:m, 0:bw], vd_s[:, 0:m], xt[:, xl:xl + bw],
                                     start=True, stop=False)
                    nc.tensor.matmul(gy[0:m, 0:bw], vd2_s[:, 0:m], xt[:, xc:xc + bw],
                                     start=False, stop=False)
                    nc.tensor.matmul(gy[0:m, 0:bw], vd_s[:, 0:m], xt[:, xr:xr + bw],
                                     start=False, stop=True)
                    nc.scalar.activation(out=sqx[0:m, c0:c0 + bw], in_=gx[0:m, 0:bw],
                                         func=ACTF.Square)
                    nc.scalar.activation(out=sqy[0:m, c0:c0 + bw], in_=gy[0:m, 0:bw],
                                         func=ACTF.Square)
                    nc.scalar.activation(out=sgx[0:m, c0:c0 + bw], in_=gx[0:m, 0:bw],
                                         func=ACTF.Sign)
                    nc.scalar.activation(out=sgy[0:m, c0:c0 + bw], in_=gy[0:m, 0:bw],
                                         func=ACTF.Sign)

                nc.vector.tensor_add(msq[0:m, 1:gw + 1], sqx[0:m, 1:gw + 1],
                                     sqy[0:m, 1:gw + 1])
                # wrap columns (jnp.roll semantics on the W axis)
                nc.vector.tensor_copy(msq[0:m, 0:1], msq[0:m, gw:gw + 1])
                nc.vector.tensor_copy(msq[0:m, mw - 1:mw], msq[0:m, 1:2])

                m1 = maskp.tile([128, mw], U8, tag="m1", name=f"m1_{sl}_{r0}")
                is90 = maskp.tile([128, mw], U8, tag="is90", name=f"i9_{sl}_{r0}")
                sm = maskp.tile([128, mw], U8, tag="sm", name=f"sm_{sl}_{r0}")
                nc.vector.scalar_tensor_tensor(
                    out=m1[0:m, 1:gw + 1], in0=sqx[0:m, 1:gw + 1], scalar=float(T2),
                    in1=sqy[0:m, 1:gw + 1], op0=ALU.mult, op1=ALU.is_le)
                nc.vector.scalar_tensor_tensor(
                    out=is90[0:m, 1:gw + 1], in0=sqy[0:m, 1:gw + 1], scalar=float(T2),
                    in1=sqx[0:m, 1:gw + 1], op0=ALU.mult, op1=ALU.is_ge)
                nc.vector.tensor_tensor(
                    out=sm[0:m, 1:gw + 1], in0=sgx[0:m, 1:gw + 1],
                    in1=sgy[0:m, 1:gw + 1], op=ALU.is_equal)

                acc = accp.tile([128, 1], F32, tag="acc", name=f"acc_{sl}_{r0}")
                nc.vector.reduce_max(acc[0:m, 0:1], msq[0:m, 1:gw + 1],
                                     axis=mybir.AxisListType.X)
                accg = accp.tile([128, 1], F32, tag="accg", name=f"ag_{sl}_{r0}")
                nc.gpsimd.partition_all_reduce(accg[0:m, 0:1], acc[0:m, 0:1],
                                               channels=m,
                                               reduce_op=bass_isa.ReduceOp.max)
                msq_t.append(msq); m1_t.append(m1); is90_t.append(is90)
                s_t.append(sm); acc_t.append(accg)

            # ---------------- thresholds ----------------
            mx = acc_t[0]
            for a in acc_t[1:]:
                nx = accp.tile([1, 1], F32, tag="mxc", name=f"mx_{sl}_{id(a)}")
                nc.vector.tensor_tensor(out=nx[0:1, 0:1], in0=mx[0:1, 0:1],
                                        in1=a[0:1, 0:1], op=ALU.max)
                mx = nx
            th1 = thrp.tile([1, 1], F32, tag="th1", name=f"th1_{sl}")
            tl1 = thrp.tile([1, 1], F32, tag="tl1", name=f"tl1_{sl}")
            nc.vector.tensor_scalar(out=th1[0:1, 0:1], in0=mx[0:1, 0:1],
                                    scalar1=float(CSQ), scalar2=None, op0=ALU.mult)
            nc.vector.tensor_scalar(out=tl1[0:1, 0:1], in0=th1[0:1, 0:1],
                                    scalar1=float(DSQ), scalar2=None, op0=ALU.mult)
            th_b = thrp.tile([128, 1], F32, tag="thb", name=f"thb_{sl}")
            tl_b = thrp.tile([128, 1], F32, tag="tlb", name=f"tlb_{sl}")
            nc.gpsimd.partition_broadcast(th_b, th1[0:1, 0:1])
            nc.gpsimd.partition_broadcast(tl_b, tl1[0:1, 0:1])

            # wrap rows (jnp.roll on H axis): first strip's row -1 <- grid row h+1;
            # last strip's row h+2 <- grid row 0
            (r0f, mf, _, _) = strips[0]
            (r0l, ml, _, _) = strips[-1]
            p_last = (h + 1) - (r0l + 1)   # partition of grid row h+1 in last strip
            p_zero = 0 - (r0f + 1)         # partition of grid row 0 in first strip
            nc.sync.dma_start(msq_t[0][0:1, :], msq_t[-1][p_last:p_last + 1, :])
            nc.sync.dma_start(msq_t[-1][p_last + 1:p_last + 2, :],
                              msq_t[0][p_zero:p_zero + 1, :])

            # ---------------- pass B ----------------
            for si, (r0, m, first, last) in enumerate(strips):
                msq, m1, is90, sm = msq_t[si], m1_t[si], is90_t[si], s_t[si]
                ut = udp.tile([128, mw], F32, tag="ut", name=f"ut_{sl}_{r0}")
                dt = udp.tile([128, mw], F32, tag="dt", name=f"dt_{sl}_{r0}")
                nc.gpsimd.memset(ut[0:1, :], 0.0)
                nc.sync.dma_start(dt[m - 1:m, :], msq[0:1, :])  # junk row, never consumed
                nc.sync.dma_start(ut[1:m, :], msq[0:m - 1, :])
                nc.sync.dma_start(dt[0:m - 1, :], msq[1:m, :])

                nh = nmaxp.tile([128, mw], F32, tag="nh", name=f"nh_{sl}_{r0}")
                nv = nmaxp.tile([128, mw], F32, tag="nv", name=f"nv_{sl}_{r0}")
                n45 = nmaxp.tile([128, mw], F32, tag="n45", name=f"n45_{sl}_{r0}")
                n135 = nmaxp.tile([128, mw], F32, tag="n135", name=f"n135_{sl}_{r0}")
                nc.vector.tensor_tensor(out=nh[0:m, 1:gw + 1], in0=msq[0:m, 0:gw],
                                        in1=msq[0:m, 2:gw + 2], op=ALU.max)
                nc.vector.tensor_tensor(out=nv[0:m, 1:gw + 1], in0=ut[0:m, 1:gw + 1],
                                        in1=dt[0:m, 1:gw + 1], op=ALU.max)
                # 45 deg: NW (up,left) & SE (down,right)
                nc.vector.tensor_tensor(out=n45[0:m, 1:gw + 1], in0=ut[0:m, 0:gw],
                                        in1=dt[0:m, 2:gw + 2], op=ALU.max)
                # 135 deg: NE (up,right) & SW (down,left)
                nc.vector.tensor_tensor(out=n135[0:m, 1:gw + 1], in0=ut[0:m, 2:gw + 2],
                                        in1=dt[0:m, 0:gw], op=ALU.max)

                nsel = selp.tile([128, mw], F32, tag="nsel", name=f"ns_{sl}_{r0}")
                ndg = selp.tile([128, mw], F32, tag="ndg", name=f"nd_{sl}_{r0}")
                nc.scalar.copy(ndg[0:m, 1:gw + 1], n135[0:m, 1:gw + 1])
                nc.vector.copy_predicated(ndg[0:m, 1:gw + 1], sm[0:m, 1:gw + 1],
                                          n45[0:m, 1:gw + 1])
                nc.scalar.copy(nsel[0:m, 1:gw + 1], nh[0:m, 1:gw + 1])
                nc.vector.copy_predicated(nsel[0:m, 1:gw + 1], m1[0:m, 1:gw + 1],
                                          ndg[0:m, 1:gw + 1])
                nc.vector.copy_predicated(nsel[0:m, 1:gw + 1], is90[0:m, 1:gw + 1],
                                          nv[0:m, 1:gw + 1])

                nth = selp.tile([128, mw], F32, tag="nth", name=f"nt_{sl}_{r0}")
                ntl = selp.tile([128, mw], F32, tag="ntl", name=f"ntl_{sl}_{r0}")
                nc.vector.tensor_scalar(out=nth[0:m, 1:gw + 1], in0=nsel[0:m, 1:gw + 1],
                                        scalar1=th_b[0:m, 0:1], scalar2=None,
                                        op0=ALU.max)
                nc.vector.tensor_scalar(out=ntl[0:m, 1:gw + 1], in0=nsel[0:m, 1:gw + 1],
                                        scalar1=tl_b[0:m, 0:1], scalar2=None,
                                        op0=ALU.max)
                strong = bfp.tile([128, mw], BF16, tag="strong", name=f"st_{sl}_{r0}")
                kb = bfp.tile([128, mw], BF16, tag="kb", name=f"kb_{sl}_{r0}")
                nc.vector.tensor_tensor(out=strong[0:m, 1:gw + 1],
                                        in0=msq[0:m, 1:gw + 1],
                                        in1=nth[0:m, 1:gw + 1], op=ALU.is_ge)
                nc.vector.tensor_tensor(out=kb[0:m, 1:gw + 1], in0=msq[0:m, 1:gw + 1],
                                        in1=ntl[0:m, 1:gw + 1], op=ALU.is_ge)
                weak = bfp.tile([128, mw], BF16, tag="weak", name=f"wk_{sl}_{r0}")
                nc.vector.tensor_tensor(out=weak[0:m, 1:gw + 1], in0=kb[0:m, 1:gw + 1],
                                        in1=strong[0:m, 1:gw + 1], op=ALU.subtract)

                su = bfp.tile([128, mw], BF16, tag="su", name=f"su_{sl}_{r0}")
                sd = bfp.tile([128, mw], BF16, tag="sd", name=f"sd_{sl}_{r0}")
                nc.gpsimd.memset(su[0:1, :], 0.0)
                nc.sync.dma_start(sd[m - 1:m, 1:gw + 1], strong[0:1, 1:gw + 1])  # junk row
                nc.sync.dma_start(su[1:m, 1:gw + 1], strong[0:m - 1, 1:gw + 1])
                nc.sync.dma_start(sd[0:m - 1, 1:gw + 1], strong[1:m, 1:gw + 1])
                vs3 = bfp.tile([128, mw], BF16, tag="vs3", name=f"v3_{sl}_{r0}")
                vsum = bfp.tile([128, mw], BF16, tag="vsum", name=f"vm_{sl}_{r0}")
                nc.vector.tensor_add(vs3[0:m, 1:gw + 1], su[0:m, 1:gw + 1],
                                     sd[0:m, 1:gw + 1])
                nc.vector.scalar_tensor_tensor(
                    out=vsum[0:m, 1:gw + 1], in0=strong[0:m, 1:gw + 1], scalar=1.0,
                    in1=vs3[0:m, 1:gw + 1], op0=ALU.mult, op1=ALU.add)
                h3 = bfp.tile([128, mw], BF16, tag="h3", name=f"h3_{sl}_{r0}")
                hsum = bfp.tile([128, mw], BF16, tag="hsum", name=f"hs_{sl}_{r0}")
                nc.vector.tensor_add(h3[0:m, 2:gw], vsum[0:m, 1:gw - 1],
                                     vsum[0:m, 3:gw + 1])
                nc.vector.scalar_tensor_tensor(
                    out=hsum[0:m, 2:gw], in0=vsum[0:m, 2:gw], scalar=1.0,
                    in1=h3[0:m, 2:gw], op0=ALU.mult, op1=ALU.add)
                pm = bfp.tile([128, mw], BF16, tag="pm", name=f"pm_{sl}_{r0}")
                nc.vector.tensor_scalar(out=pm[0:m, 2:gw], in0=hsum[0:m, 2:gw],
                                        scalar1=0.5, scalar2=None, op0=ALU.is_ge)
                t2m = bfp.tile([128, mw], BF16, tag="t2m", name=f"t2_{sl}_{r0}")
                nc.vector.tensor_mul(t2m[0:m, 2:gw], pm[0:m, 2:gw], weak[0:m, 2:gw])
                edg = bfp.tile([128, mw], BF16, tag="edg", name=f"ed_{sl}_{r0}")
                nc.vector.tensor_tensor(out=edg[0:m, 2:gw], in0=strong[0:m, 2:gw],
                                        in1=t2m[0:m, 2:gw], op=ALU.max)
                ef = outp.tile([128, mw], F32, tag="ef", name=f"ef_{sl}_{r0}")
                nc.scalar.copy(ef[0:m, 2:gw], edg[0:m, 2:gw])

                e0 = r0 + 3                      # first edge grid row of this strip
                e1 = min(h, r0 + 2 + EDGE_ROWS)  # last edge grid row
                nc.sync.dma_start(o_d[sl, e0 - 1:e1, 0:w],
                                  ef[2:2 + (e1 - e0 + 1), 2:gw])

    nc.compile()
    return nc


_NC_CACHE = {}


def _get_nc(dsh, h, w):
    key = (dsh, h, w)
    if key not in _NC_CACHE:
        _NC_CACHE[key] = build_nc(dsh, h, w)
    return _NC_CACHE[key]


def kernel(x, gk=None, sobel_x=None, sobel_y=None):
    """Full-input entry point: x [128, 512, 512] f32 -> edges [128, 512, 512] f32."""
    x = np.ascontiguousarray(np.asarray(x), dtype=np.float32)
    d = x.shape[0]
    nc = _get_nc(D_SH, x.shape[1], x.shape[2])
    vs, vsn, vd, vd2 = _stencil_mats()
    in_maps = []
    for c in range(N_CORES):
        in_maps.append({
            "x": x[c * D_SH:(c + 1) * D_SH],
            "vs": vs, "vsn": vsn, "vd": vd, "vd2": vd2,
        })
    res = bass_utils.run_bass_kernel_spmd(nc, in_maps, core_ids=list(range(N_CORES)))
    out = np.concatenate([res.results[c]["o"] for c in range(N_CORES)], axis=0)
    return out.astype(np.float32)
